# revision 1
# baseline (speedup 1.0000x reference)
"""GQA (RoPE + causal softmax) Trainium2 Bass kernel, 8-core SPMD.

Sharding: DP over batch (2) x TP over KV groups (4 quarters of heads).
Core c handles batch c//4 and head quarter c%4 (8 q-heads, 2 kv-heads).
Each core computes a partial o_proj ([S, D]); host sums 4 partials per batch.

All matmuls run in float32r (TF32-like, 1 cyc/row at N>=256).
Everything on-chip is kept in "transposed" layout (feature dim on
partitions), which makes x^T the only host-side layout prep needed.
"""

import numpy as np

import concourse.bass as bass
import concourse.mybir as mybir
import concourse.tile as tile
from concourse import bacc, bass_utils

B, S, D = 2, 2048, 2048
H, KV, HD = 32, 8, 64
REP = H // KV
SCALE = 1.0 / 8.0  # 1/sqrt(HD)

F32 = mybir.dt.float32
F32R = mybir.dt.float32r
EXP = mybir.ActivationFunctionType.Exp

NCHUNK = S // 512        # 4 sq chunks of 512
NKT = D // 128           # 16 k-tiles over D
NST = S // 128           # 16 sk/s tiles

# local head j (0..7) -> denom row
def _pairrow(j):
    return 2 * (j % 4) + (j // 4)


def _build_program():
    nc = bacc.Bacc()

    xT = nc.dram_tensor("xT", [D, S], F32R, kind="ExternalInput").ap()
    wq = nc.dram_tensor("wq", [D, 8 * HD], F32R, kind="ExternalInput").ap()
    wk = nc.dram_tensor("wk", [D, 2 * HD], F32R, kind="ExternalInput").ap()
    wv = nc.dram_tensor("wv", [D, 2 * HD], F32R, kind="ExternalInput").ap()
    wo = nc.dram_tensor("wo", [8 * HD, D], F32R, kind="ExternalInput").ap()
    cosT2 = nc.dram_tensor("cosT2", [128, S], F32, kind="ExternalInput").ap()
    sinT2m = nc.dram_tensor("sinT2m", [128, S], F32, kind="ExternalInput").ap()
    tri = nc.dram_tensor("tri", [128, 128], F32, kind="ExternalInput").ap()
    ident = nc.dram_tensor("ident", [128, 64], F32R, kind="ExternalInput").ap()
    selA = nc.dram_tensor("selA", [128, 512], F32R, kind="ExternalInput").ap()
    selB = nc.dram_tensor("selB", [128, 512], F32R, kind="ExternalInput").ap()
    onescol = nc.dram_tensor("onescol", [128, 1], F32R, kind="ExternalInput").ap()
    zblk = nc.dram_tensor("zblk", [128, 128], F32R, kind="ExternalInput").ap()
    opart = nc.dram_tensor("opart", [S, D], F32, kind="ExternalOutput").ap()

    with tile.TileContext(nc) as tc:
        with (
            tc.tile_pool(name="persist", bufs=1) as pp,
            tc.tile_pool(name="consts", bufs=1) as cp,
        ):
            # persistent SBUF: q^T/k^T, attention outputs, small constants
            qT = [pp.tile([128, S], F32R, tag=f"qT{t}", name=f"qT{t}") for t in range(4)]
            kT = pp.tile([128, S], F32R, tag="kT")
            outT = [pp.tile([128, S], F32R, tag=f"outT{t}", name=f"outT{t}") for t in range(4)]
            denomA = pp.tile([128, S], F32, tag="denomA")
            denomB = pp.tile([128, S], F32, tag="denomB")
            trib = cp.tile([128, 128], F32, tag="trib")
            identb = cp.tile([128, 64], F32R, tag="identb")
            selAb = cp.tile([128, 512], F32R, tag="selAb")
            selBb = cp.tile([128, 512], F32R, tag="selBb")
            onesb = cp.tile([128, 1], F32R, tag="onesb")
            zblkb = cp.tile([128, 128], F32R, tag="zblkb")
            nc.sync.dma_start(trib[:], tri[:])
            nc.sync.dma_start(identb[:], ident[:])
            nc.sync.dma_start(selAb[:], selA[:])
            nc.sync.dma_start(selBb[:], selB[:])
            nc.sync.dma_start(onesb[:], onescol[:])
            nc.sync.dma_start(zblkb[:], zblk[:])
            nc.gpsimd.memset(denomA[:], 1.0)
            nc.gpsimd.memset(denomB[:], 1.0)

            vo = [[None] * NST, [None] * NST]
            with tc.tile_pool(name="vop", bufs=1) as vp:  # spans phases A..D
                with (
                    tc.tile_pool(name="ropec", bufs=1) as rcc,
                    tc.tile_pool(name="vtbuf", bufs=1) as vtb,
                ):
                    cosb = rcc.tile([128, S], F32, tag="cosb")
                    sinb = rcc.tile([128, S], F32, tag="sinb")
                    nc.sync.dma_start(cosb[:], cosT2[:])
                    nc.sync.dma_start(sinb[:], sinT2m[:])
                    vT = vtb.tile([128, S], F32R, tag="vT")

                    # ---------- Phase A: qkv^T = W^T @ x^T ----------
                    with (
                        tc.tile_pool(name="wts", bufs=1) as wp,
                        tc.tile_pool(name="xin", bufs=4) as xp,
                        tc.tile_pool(name="qkvps", bufs=6, space="PSUM") as pqkv,
                    ):
                        wqk = [wp.tile([128, 8 * HD], F32R, tag=f"wq{k}", name=f"wqk{k}") for k in range(NKT)]
                        wkk = [wp.tile([128, 2 * HD], F32R, tag=f"wk{k}", name=f"wkk{k}") for k in range(NKT)]
                        wvk = [wp.tile([128, 2 * HD], F32R, tag=f"wv{k}", name=f"wvk{k}") for k in range(NKT)]
                        for k in range(NKT):
                            r = slice(k * 128, (k + 1) * 128)
                            nc.sync.dma_start(wqk[k][:], wq[r, :])
                            nc.sync.dma_start(wkk[k][:], wk[r, :])
                            nc.sync.dma_start(wvk[k][:], wv[r, :])
                        for n in range(NCHUNK):
                            ncol = slice(n * 512, (n + 1) * 512)
                            accs = [pqkv.tile([128, 512], F32, tag="qkvacc", name=f"acc{n}_{m}") for m in range(6)]
                            for k in range(NKT):
                                xk = xp.tile([128, 512], F32R, tag="xk")
                                nc.sync.dma_start(xk[:], xT[k * 128:(k + 1) * 128, ncol])
                                st = k == 0
                                sp = k == NKT - 1
                                for t in range(4):
                                    nc.tensor.matmul(
                                        accs[t][:], wqk[k][:, t * 128:(t + 1) * 128],
                                        xk[:], start=st, stop=sp)
                                nc.tensor.matmul(accs[4][:], wkk[k][:], xk[:], start=st, stop=sp)
                                nc.tensor.matmul(accs[5][:], wvk[k][:], xk[:], start=st, stop=sp)
                            for t in range(4):
                                nc.vector.tensor_copy(qT[t][:, ncol], accs[t][:])
                            nc.vector.tensor_copy(kT[:, ncol], accs[4][:])
                            nc.vector.tensor_copy(vT[:, ncol], accs[5][:])

                    # ---------- Phase B: RoPE on q^T and k^T ----------
                    with tc.tile_pool(name="rope", bufs=2) as rp:
                        for tl in [*qT, kT]:
                            rot = rp.tile([128, S], F32, tag="rot")
                            tmp = rp.tile([128, S], F32, tag="tmp")
                            # rotate-half as partition-shifted copies (sign folded in sinb)
                            nc.gpsimd.tensor_copy(rot[0:32, :], tl[32:64, :])
                            nc.gpsimd.tensor_copy(rot[32:64, :], tl[0:32, :])
                            nc.gpsimd.tensor_copy(rot[64:96, :], tl[96:128, :])
                            nc.gpsimd.tensor_copy(rot[96:128, :], tl[64:96, :])
                            nc.vector.tensor_mul(tmp[:], tl[:], cosb[:])
                            nc.vector.tensor_mul(rot[:], rot[:], sinb[:])
                            nc.vector.tensor_add(tl[:], tmp[:], rot[:])

                    # ---------- Phase C: v natural tiles [128, 65] ----------
                    with tc.tile_pool(name="vtp", bufs=2, space="PSUM") as vtp:
                        for g in range(2):
                            for i in range(NST):
                                vps = vtp.tile([128, 64], F32R, tag="vps")
                                nc.tensor.transpose(
                                    vps[:], vT[g * 64:(g + 1) * 64, i * 128:(i + 1) * 128],
                                    identb[g * 64:(g + 1) * 64, :])
                                vt = vp.tile([128, 65], F32R, tag=f"vo{g}_{i}", name=f"vo{g}_{i}")
                                nc.vector.tensor_copy(vt[:, 0:64], vps[:])
                                nc.vector.tensor_copy(vt[:, 64:65], onesb[:])
                                vo[g][i] = vt

                # ---------- Phase D: attention ----------
                with (
                    tc.tile_pool(name="esb", bufs=10) as ep,
                    tc.tile_pool(name="sps", bufs=4, space="PSUM") as sp_,
                    tc.tile_pool(name="avp", bufs=3, space="PSUM") as ap_,
                ):
                    for t in range(4):
                        for j in range(NCHUNK):
                            jcol = slice(j * 512, (j + 1) * 512)
                            avs = []
                            for sub in range(2):
                                avs.append(ap_.tile([65, 512], F32, tag="avacc", name=f"av{t}_{j}_{sub}"))
                            for i in range(4 * j + 4):
                                c0 = max(0, 128 * (i - 4 * j))
                                ec0 = c0 if 512 - c0 >= 256 else 256
                                av0 = c0 if c0 < 384 else 256
                                for sub in range(2):
                                    pb = slice(64 * sub, 64 * sub + 64)
                                    g = sub
                                    ss = sp_.tile([128, 512], F32, tag="scps")
                                    nc.tensor.matmul(
                                        ss[:, ec0:512],
                                        kT[pb, i * 128:(i + 1) * 128],
                                        qT[t][pb, j * 512 + ec0:(j + 1) * 512],
                                        start=True, stop=True)
                                    es = ep.tile([128, 512], F32R, tag="es")
                                    nc.scalar.activation(
                                        es[:, c0:512], ss[:, c0:512], EXP, scale=SCALE)
                                    if i >= 4 * j:
                                        nc.vector.tensor_mul(
                                            es[:, c0:c0 + 128], es[:, c0:c0 + 128],
                                            trib[:])
                                    if c0 == 384:
                                        nc.vector.tensor_copy(es[:, 256:384], zblkb[:])
                                    nc.tensor.matmul(
                                        avs[sub][:, av0:512], vo[g][i][:],
                                        es[:, av0:512],
                                        start=(i == 0), stop=(i == 4 * j + 3))
                            for sub in range(2):
                                pb = slice(64 * sub, 64 * sub + 64)
                                nc.vector.tensor_copy(outT[t][pb, jcol], avs[sub][0:64, :])
                                dst = denomA if sub == 0 else denomB
                                nc.vector.tensor_copy(
                                    dst[32 * t:32 * t + 1, jcol], avs[sub][64:65, :])

            # ---------- Phase E: normalize + o_proj ----------
            with (
                tc.tile_pool(name="norm", bufs=2) as np_,
                tc.tile_pool(name="wop", bufs=1) as wop,
                tc.tile_pool(name="oout", bufs=3) as op,
                tc.tile_pool(name="bcps", bufs=2, space="PSUM") as bp_,
                tc.tile_pool(name="ops", bufs=4, space="PSUM") as opp,
                tc.tile_pool(name="rcp", bufs=1) as rcp,
            ):
                rcpf = rcp.tile([128, S], F32, tag="rcpf")
                rcprA = rcp.tile([128, S], F32R, tag="rcprA")
                rcprB = rcp.tile([128, S], F32R, tag="rcprB")
                for dt_, rr in ((denomA, rcprA), (denomB, rcprB)):
                    nc.vector.reciprocal(rcpf[:], dt_[:])
                    nc.vector.tensor_copy(rr[:], rcpf[:])
                for t in range(4):
                    tsl = slice(t * 128, (t + 1) * 128)
                    bcs = np_.tile([128, S], F32, tag="bcs")
                    for n in range(NCHUNK):
                        ncol = slice(n * 512, (n + 1) * 512)
                        bps = bp_.tile([128, 512], F32, tag="bps")
                        nc.tensor.matmul(
                            bps[:], selAb[:, tsl], rcprA[:, ncol],
                            start=True, stop=False)
                        nc.tensor.matmul(
                            bps[:], selBb[:, tsl], rcprB[:, ncol],
                            start=False, stop=True)
                        nc.vector.tensor_copy(bcs[:, ncol], bps[:])
                    nc.vector.tensor_mul(outT[t][:], outT[t][:], bcs[:])
                wot = [wop.tile([128, S], F32R, tag=f"wo{k}", name=f"wot{k}") for k in range(4)]
                for k in range(4):
                    nc.sync.dma_start(wot[k][:], wo[k * 128:(k + 1) * 128, :])
                for st in range(NST):
                    for dch in range(NCHUNK):
                        ops = opp.tile([128, 512], F32, tag="opps")
                        for kt in range(4):
                            nc.tensor.matmul(
                                ops[:], outT[kt][:, st * 128:(st + 1) * 128],
                                wot[kt][:, dch * 512:(dch + 1) * 512],
                                start=(kt == 0), stop=(kt == 3))
                        oo = op.tile([128, 512], F32, tag="oo")
                        nc.vector.tensor_copy(oo[:], ops[:])
                        nc.sync.dma_start(
                            opart[st * 128:(st + 1) * 128, dch * 512:(dch + 1) * 512],
                            oo[:])

    nc.compile()
    return nc


_PROGRAM = None


def _get_program():
    global _PROGRAM
    if _PROGRAM is None:
        _PROGRAM = _build_program()
    return _PROGRAM


def _make_in_maps(x, cos, sin, Wq, Wk, Wv, Wo):
    cosT = np.ascontiguousarray(cos.T.astype(np.float32))      # [64, S]
    sinT = np.ascontiguousarray(sin.T.astype(np.float32))
    cosT2 = np.tile(cosT, (2, 1))
    sinT2m = np.tile(np.concatenate([-sinT[:32], sinT[32:]], 0), (2, 1))
    tri = (np.arange(128)[None, :] >= np.arange(128)[:, None]).astype(np.float32)
    ident = np.tile(np.eye(64, dtype=np.float32), (2, 1))
    selA = np.zeros((128, 512), dtype=np.float32)
    selB = np.zeros((128, 512), dtype=np.float32)
    for t in range(4):
        selA[32 * t, 128 * t:128 * t + 64] = 1.0
        selB[32 * t, 128 * t + 64:128 * t + 128] = 1.0

    perm = [0, 4, 1, 5, 2, 6, 3, 7]
    in_maps = []
    for c in range(8):
        b, q = c // 4, c % 4
        idx = np.concatenate([np.arange(HD) + (8 * q + j) * HD for j in perm])
        in_maps.append({
            "xT": np.ascontiguousarray(x[b].T.astype(np.float32)),
            "wq": np.ascontiguousarray(Wq[:, idx].astype(np.float32)),
            "wk": np.ascontiguousarray(Wk[:, 2 * q * HD:(2 * q + 2) * HD].astype(np.float32)),
            "wv": np.ascontiguousarray(Wv[:, 2 * q * HD:(2 * q + 2) * HD].astype(np.float32)),
            "wo": np.ascontiguousarray(Wo[idx, :].astype(np.float32)),
            "cosT2": cosT2,
            "sinT2m": sinT2m,
            "tri": tri,
            "ident": ident,
            "selA": selA,
            "selB": selB,
            "onescol": np.ones((128, 1), dtype=np.float32),
            "zblk": np.zeros((128, 128), dtype=np.float32),
        })
    return in_maps


def _execute(in_maps, trace=False):
    nc = _get_program()
    return bass_utils.run_bass_kernel_spmd(
        nc, in_maps, core_ids=list(range(8)), trace=trace)


def kernel(x, cos, sin, Wq, Wk, Wv, Wo):
    in_maps = _make_in_maps(x, cos, sin, Wq, Wk, Wv, Wo)
    res = _execute(in_maps, trace=False)
    parts = [r["opart"] for r in res.results]
    out = np.empty((B, S, D), dtype=np.float32)
    for b in range(B):
        p = parts[4 * b:4 * b + 4]
        out[b] = (p[0] + p[1]) + (p[2] + p[3])
    return out



# revision 11
# speedup vs baseline: 1.6318x; 1.6318x over previous
"""GQA (RoPE + causal softmax) Trainium2 Bass kernel, 8-core SPMD. v2.

Sharding: DP over batch (2) x TP over head quarters (4).
Core c handles batch c//4 and head quarter c%4 (8 q-heads, 2 kv-heads).
Each core computes a partial o_proj ([S, D]); host sums 4 partials per batch.

v2 design (vs v1):
- bf16 operands everywhere (inputs, weights, q/k/v, es, o_proj), f32 PSUM.
- RoPE via PE permutation-matmul + DVE combine, fused into phase A
  per-chunk (replaces catastrophically slow gpsimd partition copies).
- Attention (phase D): row-packed score matmul pairs (two 64-row heads
  concurrently via PE row tiling), two-bank [128,2,512] score tiles with
  a single wide exp per block-pair, exact causal trimming.
- o_proj + normalization pipelined per 512-chunk j right after D(j)
  (denominators for chunk j are final then) to keep PE dense.
"""

import numpy as np
import ml_dtypes

import concourse.bass as bass
import concourse.mybir as mybir
import concourse.tile as tile
from concourse import bacc, bass_utils

B, S, D = 2, 2048, 2048
H, KV, HD = 32, 8, 64
REP = H // KV
SCALE = 1.0 / 8.0  # 1/sqrt(HD)

F32 = mybir.dt.float32
BF16 = mybir.dt.bfloat16
EXP = mybir.ActivationFunctionType.Exp

NCHUNK = S // 512        # 4 sq chunks of 512
NKT = D // 128           # 16 k-tiles over D
NST = S // 128           # 16 sk/st tiles

BF = ml_dtypes.bfloat16


def _build_program():
    nc = bacc.Bacc()

    xT = nc.dram_tensor("xT", [D, S], BF16, kind="ExternalInput").ap()
    wq = nc.dram_tensor("wq", [D, 8 * HD], BF16, kind="ExternalInput").ap()
    wk = nc.dram_tensor("wk", [D, 2 * HD], BF16, kind="ExternalInput").ap()
    wv = nc.dram_tensor("wv", [D, 2 * HD], BF16, kind="ExternalInput").ap()
    wo = nc.dram_tensor("wo", [8 * HD, D], BF16, kind="ExternalInput").ap()
    cosT2 = nc.dram_tensor("cosT2", [128, S], BF16, kind="ExternalInput").ap()
    sinP = nc.dram_tensor("sinP", [128, S], BF16, kind="ExternalInput").ap()
    tri = nc.dram_tensor("tri", [128, 128], BF16, kind="ExternalInput").ap()
    permm = nc.dram_tensor("permm", [128, 128], BF16, kind="ExternalInput").ap()
    ident = nc.dram_tensor("ident", [128, 64], BF16, kind="ExternalInput").ap()
    selA = nc.dram_tensor("selA", [128, 512], BF16, kind="ExternalInput").ap()
    selB = nc.dram_tensor("selB", [128, 512], BF16, kind="ExternalInput").ap()
    opart = nc.dram_tensor("opart", [S, D], F32, kind="ExternalOutput").ap()

    with tile.TileContext(nc) as tc:
        with (
            tc.tile_pool(name="persist", bufs=1) as pp,
            tc.tile_pool(name="consts", bufs=1) as cp,
        ):
            # persistent SBUF
            qT = [pp.tile([128, S], BF16, tag=f"qT{t}", name=f"qT{t}") for t in range(4)]
            kT = pp.tile([128, S], BF16, tag="kT")
            outT = [pp.tile([128, S], BF16, tag=f"outT{t}", name=f"outT{t}") for t in range(4)]
            denomA = pp.tile([128, S], F32, tag="denomA")
            denomB = pp.tile([128, S], F32, tag="denomB")
            vo = [[None] * NST, [None] * NST]
            for g in range(2):
                for i in range(NST):
                    vt = pp.tile([128, 65], BF16, tag=f"vo{g}_{i}", name=f"vo{g}_{i}")
                    vo[g][i] = vt
            # constants
            cosb = cp.tile([128, S], BF16, tag="cosb")
            sinb = cp.tile([128, S], BF16, tag="sinb")
            trib = cp.tile([128, 128], BF16, tag="trib")
            permb = cp.tile([128, 128], BF16, tag="permb")
            identb = cp.tile([128, 64], BF16, tag="identb")
            selAb = cp.tile([128, 512], BF16, tag="selAb")
            selBb = cp.tile([128, 512], BF16, tag="selBb")
            nc.sync.dma_start(cosb[:], cosT2[:])
            nc.sync.dma_start(sinb[:], sinP[:])
            nc.sync.dma_start(trib[:], tri[:])
            nc.sync.dma_start(permb[:], permm[:])
            nc.sync.dma_start(identb[:], ident[:])
            nc.sync.dma_start(selAb[:], selA[:])
            nc.sync.dma_start(selBb[:], selB[:])
            nc.gpsimd.memset(denomA[:], 1.0)
            nc.gpsimd.memset(denomB[:], 1.0)
            for g in range(2):
                for i in range(NST):
                    nc.vector.memset(vo[g][i][:, 64:65], 1.0)

            # ---------- Phase A: qkv projections + RoPE + v transpose ----------
            with (
                tc.tile_pool(name="wts", bufs=1) as wp,
                tc.tile_pool(name="xin", bufs=20) as xp,
                tc.tile_pool(name="aps", bufs=3, space="PSUM") as pa,
                tc.tile_pool(name="rps", bufs=2, space="PSUM") as pr,
                tc.tile_pool(name="vps", bufs=2, space="PSUM") as pv,
                tc.tile_pool(name="ropes", bufs=4) as rs,
                tc.tile_pool(name="vsb", bufs=2) as vs,
            ):
                wqk = [wp.tile([128, 8 * HD], BF16, tag=f"wq{k}", name=f"wqk{k}") for k in range(NKT)]
                wkk = [wp.tile([128, 2 * HD], BF16, tag=f"wk{k}", name=f"wkk{k}") for k in range(NKT)]
                wvk = [wp.tile([128, 2 * HD], BF16, tag=f"wv{k}", name=f"wvk{k}") for k in range(NKT)]
                for k in range(NKT):
                    r = slice(k * 128, (k + 1) * 128)
                    nc.sync.dma_start(wqk[k][:], wq[r, :])
                    nc.sync.dma_start(wkk[k][:], wk[r, :])
                    nc.sync.dma_start(wvk[k][:], wv[r, :])

                for n in range(NCHUNK):
                    ncol = slice(n * 512, (n + 1) * 512)
                    xk = []
                    for k in range(NKT):
                        xt = xp.tile([128, 512], BF16, tag="xk", name=f"xk{n}_{k}")
                        nc.sync.dma_start(xt[:], xT[k * 128:(k + 1) * 128, ncol])
                        xk.append(xt)
                    # two passes of 3 accumulators each
                    # pass A: q tiles 0,1,2 ; pass B: q tile 3, k, v
                    accsA = [pa.tile([128, 512], F32, tag="acc", name=f"accA{n}_{m}") for m in range(3)]
                    for k in range(NKT):
                        st, sp = k == 0, k == NKT - 1
                        for m in range(3):
                            nc.tensor.matmul(
                                accsA[m][:], wqk[k][:, m * 128:(m + 1) * 128],
                                xk[k][:], start=st, stop=sp)
                    accsB = [pa.tile([128, 512], F32, tag="acc", name=f"accB{n}_{m}") for m in range(3)]
                    for k in range(NKT):
                        st, sp = k == 0, k == NKT - 1
                        nc.tensor.matmul(accsB[0][:], wqk[k][:, 384:512], xk[k][:], start=st, stop=sp)
                        nc.tensor.matmul(accsB[1][:], wkk[k][:], xk[k][:], start=st, stop=sp)
                        nc.tensor.matmul(accsB[2][:], wvk[k][:], xk[k][:], start=st, stop=sp)

                    # RoPE for q0..q3 and k: dst = (acc*cos) + Pm @ (acc*sinP)
                    rope_list = [
                        (qT[0], accsA[0]), (qT[1], accsA[1]), (qT[2], accsA[2]),
                        (qT[3], accsB[0]), (kT, accsB[1]),
                    ]
                    for dst, acc in rope_list:
                        u = rs.tile([128, 512], BF16, tag="u")
                        tq = rs.tile([128, 512], BF16, tag="tq")
                        nc.vector.tensor_mul(u[:], acc[:], sinb[:, ncol])
                        nc.vector.tensor_mul(tq[:], acc[:], cosb[:, ncol])
                        rp = pr.tile([128, 512], F32, tag="rp")
                        nc.tensor.matmul(rp[:], permb[:], u[:], start=True, stop=True)
                        nc.vector.tensor_add(dst[:, ncol], tq[:], rp[:])

                    # v: evacuate + transpose into vo tiles
                    vc = vs.tile([128, 512], BF16, tag="vc")
                    nc.vector.tensor_copy(vc[:], accsB[2][:])
                    for g in range(2):
                        for blk in range(4):
                            vps = pv.tile([128, 64], BF16, tag="vps")
                            nc.tensor.transpose(
                                vps[:], vc[g * 64:(g + 1) * 64, blk * 128:(blk + 1) * 128],
                                identb[g * 64:(g + 1) * 64, :])
                            nc.vector.tensor_copy(vo[g][4 * n + blk][:, 0:64], vps[:])

            # ---------- Phase D+E: attention, normalize, o_proj per chunk ----------
            with (
                tc.tile_pool(name="wop", bufs=1) as wop,
                tc.tile_pool(name="ssp", bufs=2, space="PSUM") as sp_,
                tc.tile_pool(name="avp", bufs=2, space="PSUM") as ap_,
                tc.tile_pool(name="opp", bufs=2, space="PSUM") as op_,
                tc.tile_pool(name="esb", bufs=6) as ep,
                tc.tile_pool(name="rcb", bufs=2) as rb,
                tc.tile_pool(name="oob", bufs=3) as ob,
            ):
                wot = [wop.tile([128, S], BF16, tag=f"wo{k}", name=f"wot{k}") for k in range(4)]
                for k in range(4):
                    nc.sync.dma_start(wot[k][:], wo[k * 128:(k + 1) * 128, :])

                for j in range(NCHUNK):
                    jcol = slice(j * 512, (j + 1) * 512)
                    for t in range(4):
                        avs = [
                            ap_.tile([65, 512], F32, tag="av", name=f"av{t}_{j}_{s_}")
                            for s_ in range(2)
                        ]
                        nblk = 4 * j + 4
                        for i in range(nblk):
                            c0 = max(0, 128 * (i - 4 * j))
                            icol = slice(i * 128, (i + 1) * 128)
                            qcol = slice(j * 512 + c0, (j + 1) * 512)
                            ss = sp_.tile([128, 2, 512], F32, tag="ss")
                            nc.tensor.matmul(
                                ss[:, 0:1, c0:512], kT[0:64, icol], qT[t][0:64, qcol],
                                start=True, stop=True)
                            nc.tensor.matmul(
                                ss[:, 1:2, c0:512], kT[64:128, icol], qT[t][64:128, qcol],
                                start=True, stop=True)
                            es = ep.tile([128, 2, 512], BF16, tag="es")
                            nc.scalar.activation(
                                es[:, :, c0:512], ss[:, :, c0:512], EXP, scale=SCALE)
                            if i >= 4 * j:
                                for s_ in range(2):
                                    nc.vector.tensor_mul(
                                        es[:, s_:s_ + 1, c0:c0 + 128],
                                        es[:, s_:s_ + 1, c0:c0 + 128],
                                        trib[:])
                            for s_ in range(2):
                                nc.tensor.matmul(
                                    avs[s_][:, c0:512], vo[s_][i],
                                    es[:, s_:s_ + 1, c0:512],
                                    start=(i == 0), stop=(i == nblk - 1))
                        for s_ in range(2):
                            nc.vector.tensor_copy(
                                outT[t][64 * s_:64 * s_ + 64, jcol], avs[s_][0:64, :])
                            dn = denomA if s_ == 0 else denomB
                            nc.vector.tensor_copy(
                                dn[32 * t:32 * t + 1, jcol], avs[s_][64:65, :])

                    # normalization for chunk j (denominators final now)
                    rcpbs = []
                    for dn in (denomA, denomB):
                        rcpf = rb.tile([128, 512], F32, tag="rcpf")
                        rcpb = rb.tile([128, 512], BF16, tag="rcpb")
                        nc.vector.reciprocal(rcpf[:], dn[:, jcol])
                        nc.vector.tensor_copy(rcpb[:], rcpf[:])
                        rcpbs.append(rcpb)
                    for t in range(4):
                        tsl = slice(t * 128, (t + 1) * 128)
                        bcs = op_.tile([128, 512], F32, tag="op", name=f"bcs{j}_{t}")
                        nc.tensor.matmul(
                            bcs[:], selAb[:, tsl], rcpbs[0][:], start=True, stop=False)
                        nc.tensor.matmul(
                            bcs[:], selBb[:, tsl], rcpbs[1][:], start=False, stop=True)
                        nc.vector.tensor_mul(outT[t][:, jcol], outT[t][:, jcol], bcs[:])

                    # o_proj for chunk j
                    for st in range(4 * j, 4 * j + 4):
                        stcol = slice(st * 128, (st + 1) * 128)
                        for dch in range(NCHUNK):
                            ops = op_.tile([128, 512], F32, tag="op", name=f"ops{st}_{dch}")
                            for kt in range(4):
                                nc.tensor.matmul(
                                    ops[:], outT[kt][:, stcol],
                                    wot[kt][:, dch * 512:(dch + 1) * 512],
                                    start=(kt == 0), stop=(kt == 3))
                            oo = ob.tile([128, 512], F32, tag="oo")
                            nc.vector.tensor_copy(oo[:], ops[:])
                            nc.sync.dma_start(
                                opart[stcol, dch * 512:(dch + 1) * 512], oo[:])

    nc.compile()
    return nc


_PROGRAM = None


def _get_program():
    global _PROGRAM
    if _PROGRAM is None:
        _PROGRAM = _build_program()
    return _PROGRAM


def _make_in_maps(x, cos, sin, Wq, Wk, Wv, Wo):
    x = np.asarray(x, np.float32)
    cos = np.asarray(cos, np.float32)
    sin = np.asarray(sin, np.float32)
    Wq, Wk, Wv, Wo = (np.asarray(w, np.float32) for w in (Wq, Wk, Wv, Wo))

    cosT = cos.T  # [64, S]
    sinT = sin.T
    cosT2 = np.tile(cosT, (2, 1)).astype(BF)
    # s''[i] = sign(sigma(i)) * sin[sigma(i)], sigma = xor-32 within each 64
    stld = np.concatenate([-sinT[:32], sinT[32:]], 0)          # s~ (signed sin)
    sPrm = np.concatenate([stld[32:64], stld[0:32]], 0)        # s'' = s~ . sigma
    sinP = np.tile(sPrm, (2, 1)).astype(BF)
    tri = (np.arange(128)[None, :] >= np.arange(128)[:, None]).astype(BF)
    # permutation matmul matrix: out[p] = u[sigma(p)];  Pm[r, p] = 1 iff r = sigma(p)
    sig = np.arange(128)
    sig = (sig // 64) * 64 + ((sig % 64) + 32) % 64
    permm = np.zeros((128, 128), np.float32)
    permm[sig, np.arange(128)] = 1.0
    permm = permm.astype(BF)
    ident = np.tile(np.eye(64, dtype=np.float32), (2, 1)).astype(BF)
    # selectors: bcs[p<64, s] = rcpA[32t, s]; bcs[p>=64, s] = rcpB[32t, s]
    selA = np.zeros((128, 512), np.float32)
    selB = np.zeros((128, 512), np.float32)
    for t in range(4):
        selA[32 * t, 128 * t:128 * t + 64] = 1.0
        selB[32 * t, 128 * t + 64:128 * t + 128] = 1.0
    selA = selA.astype(BF)
    selB = selB.astype(BF)

    perm = [0, 4, 1, 5, 2, 6, 3, 7]
    in_maps = []
    for c in range(8):
        b, q = c // 4, c % 4
        idx = np.concatenate([np.arange(HD) + (8 * q + j) * HD for j in perm])
        in_maps.append({
            "xT": np.ascontiguousarray(x[b].T).astype(BF),
            "wq": np.ascontiguousarray(Wq[:, idx]).astype(BF),
            "wk": np.ascontiguousarray(Wk[:, 2 * q * HD:(2 * q + 2) * HD]).astype(BF),
            "wv": np.ascontiguousarray(Wv[:, 2 * q * HD:(2 * q + 2) * HD]).astype(BF),
            "wo": np.ascontiguousarray(Wo[idx, :]).astype(BF),
            "cosT2": cosT2,
            "sinP": sinP,
            "tri": tri,
            "permm": permm,
            "ident": ident,
            "selA": selA,
            "selB": selB,
        })
    return in_maps


def _execute(in_maps, trace=False):
    nc = _get_program()
    return bass_utils.run_bass_kernel_spmd(
        nc, in_maps, core_ids=list(range(8)), trace=trace)


def kernel(x, cos, sin, Wq, Wk, Wv, Wo):
    in_maps = _make_in_maps(x, cos, sin, Wq, Wk, Wv, Wo)
    res = _execute(in_maps, trace=False)
    parts = [r["opart"] for r in res.results]
    out = np.empty((B, S, D), dtype=np.float32)
    for b in range(B):
        p = parts[4 * b:4 * b + 4]
        out[b] = (p[0] + p[1]) + (p[2] + p[3])
    return out


# revision 17
# speedup vs baseline: 2.1688x; 1.3291x over previous
"""GQA (RoPE + causal softmax) Trainium2 Bass kernel, 8-core SPMD. v2.

Sharding: DP over batch (2) x TP over head quarters (4).
Core c handles batch c//4 and head quarter c%4 (8 q-heads, 2 kv-heads).
Each core computes a partial o_proj ([S, D]); host sums 4 partials per batch.

v2 design (vs v1):
- bf16 operands everywhere (inputs, weights, q/k/v, es, o_proj), f32 PSUM.
- RoPE via PE permutation-matmul + DVE combine, fused into phase A
  per-chunk (replaces catastrophically slow gpsimd partition copies).
- Attention (phase D): row-packed score matmul pairs (two 64-row heads
  concurrently via PE row tiling), two-bank [128,2,512] score tiles with
  a single wide exp per block-pair, exact causal trimming.
- o_proj + normalization pipelined per 512-chunk j right after D(j)
  (denominators for chunk j are final then) to keep PE dense.
"""

import numpy as np
import ml_dtypes

import concourse.bass as bass
import concourse.mybir as mybir
import concourse.tile as tile
from concourse import bacc, bass_utils

B, S, D = 2, 2048, 2048
H, KV, HD = 32, 8, 64
REP = H // KV
SCALE = 1.0 / 8.0  # 1/sqrt(HD)

F32 = mybir.dt.float32
BF16 = mybir.dt.bfloat16
EXP = mybir.ActivationFunctionType.Exp

NCHUNK = S // 512        # 4 sq chunks of 512
NKT = D // 128           # 16 k-tiles over D
NST = S // 128           # 16 sk/st tiles

BF = ml_dtypes.bfloat16


def _build_program():
    nc = bacc.Bacc()

    xT = nc.dram_tensor("xT", [D, S], BF16, kind="ExternalInput").ap()
    wq = nc.dram_tensor("wq", [D, 8 * HD], BF16, kind="ExternalInput").ap()
    wk = nc.dram_tensor("wk", [D, 2 * HD], BF16, kind="ExternalInput").ap()
    wv = nc.dram_tensor("wv", [D, 2 * HD], BF16, kind="ExternalInput").ap()
    wo = nc.dram_tensor("wo", [8 * HD, D], BF16, kind="ExternalInput").ap()
    cosT2 = nc.dram_tensor("cosT2", [128, S], BF16, kind="ExternalInput").ap()
    sinP = nc.dram_tensor("sinP", [128, S], BF16, kind="ExternalInput").ap()
    tri = nc.dram_tensor("tri", [128, 128], BF16, kind="ExternalInput").ap()
    permm = nc.dram_tensor("permm", [128, 128], BF16, kind="ExternalInput").ap()
    ident = nc.dram_tensor("ident", [128, 64], BF16, kind="ExternalInput").ap()
    selA = nc.dram_tensor("selA", [128, 512], BF16, kind="ExternalInput").ap()
    selB = nc.dram_tensor("selB", [128, 512], BF16, kind="ExternalInput").ap()
    opart = nc.dram_tensor("opart", [S, D], F32, kind="ExternalOutput").ap()

    with tile.TileContext(nc) as tc:
        with (
            tc.tile_pool(name="persist", bufs=1) as pp,
            tc.tile_pool(name="consts", bufs=1) as cp,
        ):
            # persistent SBUF
            qT = [pp.tile([128, S], BF16, tag=f"qT{t}", name=f"qT{t}") for t in range(4)]
            kT = pp.tile([128, S], BF16, tag="kT")
            outT = [pp.tile([128, S], BF16, tag=f"outT{t}", name=f"outT{t}") for t in range(4)]
            denomA = pp.tile([128, S], F32, tag="denomA")
            denomB = pp.tile([128, S], F32, tag="denomB")
            vo = [[None] * NST, [None] * NST]
            for g in range(2):
                for i in range(NST):
                    vt = pp.tile([128, 65], BF16, tag=f"vo{g}_{i}", name=f"vo{g}_{i}")
                    vo[g][i] = vt
            # constants
            cosb = cp.tile([128, S], BF16, tag="cosb")
            sinb = cp.tile([128, S], BF16, tag="sinb")
            trib = cp.tile([128, 128], BF16, tag="trib")
            permb = cp.tile([128, 128], BF16, tag="permb")
            identb = cp.tile([128, 64], BF16, tag="identb")
            selAb = cp.tile([128, 512], BF16, tag="selAb")
            selBb = cp.tile([128, 512], BF16, tag="selBb")
            nc.sync.dma_start(cosb[:], cosT2[:])
            nc.sync.dma_start(sinb[:], sinP[:])
            nc.sync.dma_start(trib[:], tri[:])
            nc.sync.dma_start(permb[:], permm[:])
            nc.sync.dma_start(identb[:], ident[:])
            nc.sync.dma_start(selAb[:], selA[:])
            nc.sync.dma_start(selBb[:], selB[:])
            nc.gpsimd.memset(denomA[:], 1.0)
            nc.gpsimd.memset(denomB[:], 1.0)
            for g in range(2):
                for i in range(NST):
                    nc.vector.memset(vo[g][i][:, 64:65], 1.0)

            # ---------- Phase A: qkv projections + RoPE + v transpose ----------
            with (
                tc.tile_pool(name="wts", bufs=1) as wp,
                tc.tile_pool(name="xin", bufs=2) as xp,
                tc.tile_pool(name="aps", bufs=3, space="PSUM") as pa,
                tc.tile_pool(name="rps", bufs=2, space="PSUM") as pr,
                tc.tile_pool(name="vps", bufs=2, space="PSUM") as pv,
                tc.tile_pool(name="ropes", bufs=4) as rs,
                tc.tile_pool(name="vsb", bufs=2) as vs,
            ):
                wqa = wp.tile([128, NKT, 8 * HD], BF16, tag="wqa")
                wka = wp.tile([128, NKT, 2 * HD], BF16, tag="wka")
                wva = wp.tile([128, NKT, 2 * HD], BF16, tag="wva")
                nc.sync.dma_start(wqa[:], wq.rearrange("(k p) c -> p k c", p=128))
                nc.sync.dma_start(wka[:], wk.rearrange("(k p) c -> p k c", p=128))
                nc.sync.dma_start(wva[:], wv.rearrange("(k p) c -> p k c", p=128))
                wqk = [wqa[:, k:k + 1, :] for k in range(NKT)]
                wkk = [wka[:, k:k + 1, :] for k in range(NKT)]
                wvk = [wva[:, k:k + 1, :] for k in range(NKT)]

                for n in range(NCHUNK):
                    ncol = slice(n * 512, (n + 1) * 512)
                    xa = xp.tile([128, NKT, 512], BF16, tag="xa", name=f"xa{n}")
                    nc.sync.dma_start(
                        xa[:], xT[:, ncol].rearrange("(k p) c -> p k c", p=128))
                    xk = [xa[:, k:k + 1, :] for k in range(NKT)]
                    # two passes of 3 accumulators each
                    # pass A: q tiles 0,1,2 ; pass B: q tile 3, k, v
                    accsA = [pa.tile([128, 512], F32, tag="acc", name=f"accA{n}_{m}") for m in range(3)]
                    for k in range(NKT):
                        st, sp = k == 0, k == NKT - 1
                        for m in range(3):
                            nc.tensor.matmul(
                                accsA[m][:], wqk[k][:, :, m * 128:(m + 1) * 128],
                                xk[k], start=st, stop=sp)
                    accsB = [pa.tile([128, 512], F32, tag="acc", name=f"accB{n}_{m}") for m in range(3)]
                    for k in range(NKT):
                        st, sp = k == 0, k == NKT - 1
                        nc.tensor.matmul(accsB[0][:], wqk[k][:, :, 384:512], xk[k], start=st, stop=sp)
                        nc.tensor.matmul(accsB[1][:], wkk[k], xk[k], start=st, stop=sp)
                        nc.tensor.matmul(accsB[2][:], wvk[k], xk[k], start=st, stop=sp)

                    # RoPE for q0..q3 and k: dst = (acc*cos) + Pm @ (acc*sinP)
                    rope_list = [
                        (qT[0], accsA[0]), (qT[1], accsA[1]), (qT[2], accsA[2]),
                        (qT[3], accsB[0]), (kT, accsB[1]),
                    ]
                    for dst, acc in rope_list:
                        u = rs.tile([128, 512], BF16, tag="u")
                        tq = rs.tile([128, 512], BF16, tag="tq")
                        nc.vector.tensor_mul(u[:], acc[:], sinb[:, ncol])
                        nc.vector.tensor_mul(tq[:], acc[:], cosb[:, ncol])
                        rp = pr.tile([128, 512], F32, tag="rp")
                        nc.tensor.matmul(rp[:], permb[:], u[:], start=True, stop=True)
                        nc.vector.tensor_add(dst[:, ncol], tq[:], rp[:])

                    # v: evacuate + transpose into vo tiles
                    vc = vs.tile([128, 512], BF16, tag="vc")
                    nc.vector.tensor_copy(vc[:], accsB[2][:])
                    for g in range(2):
                        for blk in range(4):
                            vps = pv.tile([128, 64], BF16, tag="vps")
                            nc.tensor.transpose(
                                vps[:], vc[g * 64:(g + 1) * 64, blk * 128:(blk + 1) * 128],
                                identb[g * 64:(g + 1) * 64, :])
                            nc.vector.tensor_copy(vo[g][4 * n + blk][:, 0:64], vps[:])

            # ---------- Phase D+E: attention, normalize, o_proj per chunk ----------
            with (
                tc.tile_pool(name="wop", bufs=1) as wop,
                tc.tile_pool(name="ssp", bufs=2, space="PSUM") as sp_,
                tc.tile_pool(name="avp", bufs=2, space="PSUM") as ap_,
                tc.tile_pool(name="opp", bufs=2, space="PSUM") as op_,
                tc.tile_pool(name="esb", bufs=6) as ep,
                tc.tile_pool(name="rcb", bufs=2) as rb,
                tc.tile_pool(name="oob", bufs=3) as ob,
            ):
                woa = wop.tile([128, 4, S], BF16, tag="woa")
                nc.sync.dma_start(woa[:], wo.rearrange("(k p) c -> p k c", p=128))

                def d_group(t, j):
                    jcol = slice(j * 512, (j + 1) * 512)
                    avs = [
                        ap_.tile([65, 512], F32, tag="av", name=f"av{t}_{j}_{s_}")
                        for s_ in range(2)
                    ]
                    nblk = 4 * j + 4
                    for i in range(nblk):
                        c0 = max(0, 128 * (i - 4 * j))
                        icol = slice(i * 128, (i + 1) * 128)
                        qcol = slice(j * 512 + c0, (j + 1) * 512)
                        ss = sp_.tile([128, 2, 512], F32, tag="ss", name=f"ss{t}_{j}_{i}")
                        nc.tensor.matmul(
                            ss[:, 0:1, c0:512], kT[0:64, icol], qT[t][0:64, qcol],
                            start=True, stop=True)
                        nc.tensor.matmul(
                            ss[:, 1:2, c0:512], kT[64:128, icol], qT[t][64:128, qcol],
                            start=True, stop=True)
                        es = ep.tile([128, 2, 512], BF16, tag="es", name=f"es{t}_{j}_{i}")
                        nc.scalar.activation(
                            es[:, :, c0:512], ss[:, :, c0:512], EXP, scale=SCALE)
                        if i >= 4 * j:
                            for s_ in range(2):
                                nc.vector.tensor_mul(
                                    es[:, s_:s_ + 1, c0:c0 + 128],
                                    es[:, s_:s_ + 1, c0:c0 + 128],
                                    trib[:])
                        for s_ in range(2):
                            nc.tensor.matmul(
                                avs[s_][:, c0:512], vo[s_][i],
                                es[:, s_:s_ + 1, c0:512],
                                start=(i == 0), stop=(i == nblk - 1))
                    for s_ in range(2):
                        nc.vector.tensor_copy(
                            outT[t][64 * s_:64 * s_ + 64, jcol], avs[s_][0:64, :])
                        dn = denomA if s_ == 0 else denomB
                        nc.vector.tensor_copy(
                            dn[32 * t:32 * t + 1, jcol], avs[s_][64:65, :])

                def e_chunk(j):
                    jcol = slice(j * 512, (j + 1) * 512)
                    # normalization for chunk j (denominators final now)
                    rcpbs = []
                    for dn in (denomA, denomB):
                        rcpf = rb.tile([128, 512], F32, tag="rcpf")
                        rcpb = rb.tile([128, 512], BF16, tag="rcpb")
                        nc.vector.reciprocal(rcpf[:], dn[:, jcol])
                        nc.vector.tensor_copy(rcpb[:], rcpf[:])
                        rcpbs.append(rcpb)
                    for t in range(4):
                        tsl = slice(t * 128, (t + 1) * 128)
                        bcs = op_.tile([128, 512], F32, tag="op", name=f"bcs{j}_{t}")
                        nc.tensor.matmul(
                            bcs[:], selAb[:, tsl], rcpbs[0][:], start=True, stop=False)
                        nc.tensor.matmul(
                            bcs[:], selBb[:, tsl], rcpbs[1][:], start=False, stop=True)
                        nc.vector.tensor_mul(outT[t][:, jcol], outT[t][:, jcol], bcs[:])
                    # o_proj for chunk j
                    for st in range(4 * j, 4 * j + 4):
                        stcol = slice(st * 128, (st + 1) * 128)
                        oo3 = ob.tile([128, 4, 512], F32, tag="oo", name=f"oo{st}")
                        for dch in range(NCHUNK):
                            ops = op_.tile([128, 512], F32, tag="op", name=f"ops{st}_{dch}")
                            for kt in range(4):
                                nc.tensor.matmul(
                                    ops[:], outT[kt][:, stcol],
                                    woa[:, kt:kt + 1, dch * 512:(dch + 1) * 512],
                                    start=(kt == 0), stop=(kt == 3))
                            nc.vector.tensor_copy(oo3[:, dch:dch + 1, :], ops[:])
                        nc.sync.dma_start(opart[stcol, :], oo3[:])

                # emission order: stagger e(j) after the first d-group of j+1
                emission = []
                for j in range(NCHUNK):
                    emission.extend(("d", t, j) for t in range(4))
                    emission.append(("e", j))
                for idx in range(len(emission) - 1):
                    if emission[idx][0] == "e" and emission[idx + 1][0] == "d":
                        emission[idx], emission[idx + 1] = emission[idx + 1], emission[idx]
                for step in emission:
                    if step[0] == "d":
                        d_group(step[1], step[2])
                    else:
                        e_chunk(step[1])

    nc.compile()
    return nc


_PROGRAM = None


def _get_program():
    global _PROGRAM
    if _PROGRAM is None:
        _PROGRAM = _build_program()
    return _PROGRAM


def _make_in_maps(x, cos, sin, Wq, Wk, Wv, Wo):
    x = np.asarray(x, np.float32)
    cos = np.asarray(cos, np.float32)
    sin = np.asarray(sin, np.float32)
    Wq, Wk, Wv, Wo = (np.asarray(w, np.float32) for w in (Wq, Wk, Wv, Wo))

    cosT = cos.T  # [64, S]
    sinT = sin.T
    cosT2 = np.tile(cosT, (2, 1)).astype(BF)
    # s''[i] = sign(sigma(i)) * sin[sigma(i)], sigma = xor-32 within each 64
    stld = np.concatenate([-sinT[:32], sinT[32:]], 0)          # s~ (signed sin)
    sPrm = np.concatenate([stld[32:64], stld[0:32]], 0)        # s'' = s~ . sigma
    sinP = np.tile(sPrm, (2, 1)).astype(BF)
    tri = (np.arange(128)[None, :] >= np.arange(128)[:, None]).astype(BF)
    # permutation matmul matrix: out[p] = u[sigma(p)];  Pm[r, p] = 1 iff r = sigma(p)
    sig = np.arange(128)
    sig = (sig // 64) * 64 + ((sig % 64) + 32) % 64
    permm = np.zeros((128, 128), np.float32)
    permm[sig, np.arange(128)] = 1.0
    permm = permm.astype(BF)
    ident = np.tile(np.eye(64, dtype=np.float32), (2, 1)).astype(BF)
    # selectors: bcs[p<64, s] = rcpA[32t, s]; bcs[p>=64, s] = rcpB[32t, s]
    selA = np.zeros((128, 512), np.float32)
    selB = np.zeros((128, 512), np.float32)
    for t in range(4):
        selA[32 * t, 128 * t:128 * t + 64] = 1.0
        selB[32 * t, 128 * t + 64:128 * t + 128] = 1.0
    selA = selA.astype(BF)
    selB = selB.astype(BF)

    perm = [0, 4, 1, 5, 2, 6, 3, 7]
    in_maps = []
    for c in range(8):
        b, q = c // 4, c % 4
        idx = np.concatenate([np.arange(HD) + (8 * q + j) * HD for j in perm])
        in_maps.append({
            "xT": np.ascontiguousarray(x[b].T).astype(BF),
            "wq": np.ascontiguousarray(Wq[:, idx]).astype(BF),
            "wk": np.ascontiguousarray(Wk[:, 2 * q * HD:(2 * q + 2) * HD]).astype(BF),
            "wv": np.ascontiguousarray(Wv[:, 2 * q * HD:(2 * q + 2) * HD]).astype(BF),
            "wo": np.ascontiguousarray(Wo[idx, :]).astype(BF),
            "cosT2": cosT2,
            "sinP": sinP,
            "tri": tri,
            "permm": permm,
            "ident": ident,
            "selA": selA,
            "selB": selB,
        })
    return in_maps


def _execute(in_maps, trace=False):
    nc = _get_program()
    return bass_utils.run_bass_kernel_spmd(
        nc, in_maps, core_ids=list(range(8)), trace=trace)


def kernel(x, cos, sin, Wq, Wk, Wv, Wo):
    in_maps = _make_in_maps(x, cos, sin, Wq, Wk, Wv, Wo)
    res = _execute(in_maps, trace=False)
    parts = [r["opart"] for r in res.results]
    out = np.empty((B, S, D), dtype=np.float32)
    for b in range(B):
        p = parts[4 * b:4 * b + 4]
        out[b] = (p[0] + p[1]) + (p[2] + p[3])
    return out


# revision 20
# speedup vs baseline: 2.3448x; 1.0812x over previous
"""GQA (RoPE + causal softmax) Trainium2 Bass kernel, 8-core SPMD. v2.

Sharding: DP over batch (2) x TP over head quarters (4).
Core c handles batch c//4 and head quarter c%4 (8 q-heads, 2 kv-heads).
Each core computes a partial o_proj ([S, D]); host sums 4 partials per batch.

v2 design (vs v1):
- bf16 operands everywhere (inputs, weights, q/k/v, es, o_proj), f32 PSUM.
- RoPE via PE permutation-matmul + DVE combine, fused into phase A
  per-chunk (replaces catastrophically slow gpsimd partition copies).
- Attention (phase D): row-packed score matmul pairs (two 64-row heads
  concurrently via PE row tiling), two-bank [128,2,512] score tiles with
  a single wide exp per block-pair, exact causal trimming.
- o_proj + normalization pipelined per 512-chunk j right after D(j)
  (denominators for chunk j are final then) to keep PE dense.
"""

import numpy as np
import ml_dtypes

import concourse.bass as bass
import concourse.mybir as mybir
import concourse.tile as tile
from concourse import bacc, bass_utils

B, S, D = 2, 2048, 2048
H, KV, HD = 32, 8, 64
REP = H // KV
SCALE = 1.0 / 8.0  # 1/sqrt(HD)

F32 = mybir.dt.float32
BF16 = mybir.dt.bfloat16
EXP = mybir.ActivationFunctionType.Exp

NCHUNK = S // 512        # 4 sq chunks of 512
NKT = D // 128           # 16 k-tiles over D
NST = S // 128           # 16 sk/st tiles

BF = ml_dtypes.bfloat16


def _build_program():
    nc = bacc.Bacc()

    xT = nc.dram_tensor("xT", [D, S], BF16, kind="ExternalInput").ap()
    wq = nc.dram_tensor("wq", [D, 8 * HD], BF16, kind="ExternalInput").ap()
    wk = nc.dram_tensor("wk", [D, 2 * HD], BF16, kind="ExternalInput").ap()
    wv = nc.dram_tensor("wv", [D, 2 * HD], BF16, kind="ExternalInput").ap()
    wo = nc.dram_tensor("wo", [8 * HD, D], BF16, kind="ExternalInput").ap()
    cosT2 = nc.dram_tensor("cosT2", [128, S], BF16, kind="ExternalInput").ap()
    sinP = nc.dram_tensor("sinP", [128, S], BF16, kind="ExternalInput").ap()
    tri = nc.dram_tensor("tri", [128, 128], BF16, kind="ExternalInput").ap()
    permm = nc.dram_tensor("permm", [128, 128], BF16, kind="ExternalInput").ap()
    ident = nc.dram_tensor("ident", [128, 64], BF16, kind="ExternalInput").ap()
    selA = nc.dram_tensor("selA", [128, 512], BF16, kind="ExternalInput").ap()
    selB = nc.dram_tensor("selB", [128, 512], BF16, kind="ExternalInput").ap()
    opart = nc.dram_tensor("opart", [S, D], F32, kind="ExternalOutput").ap()

    with tile.TileContext(nc) as tc:
        with (
            tc.tile_pool(name="persist", bufs=1) as pp,
            tc.tile_pool(name="consts", bufs=1) as cp,
        ):
            # persistent SBUF
            qT = [pp.tile([128, S], BF16, tag=f"qT{t}", name=f"qT{t}") for t in range(4)]
            kT = pp.tile([128, S], BF16, tag="kT")
            outT = [pp.tile([128, S], BF16, tag=f"outT{t}", name=f"outT{t}") for t in range(4)]
            denomA = pp.tile([128, S], F32, tag="denomA")
            denomB = pp.tile([128, S], F32, tag="denomB")
            vo = [[None] * NST, [None] * NST]
            for g in range(2):
                for i in range(NST):
                    vt = pp.tile([128, 65], BF16, tag=f"vo{g}_{i}", name=f"vo{g}_{i}")
                    vo[g][i] = vt
            # constants
            cosb = cp.tile([128, S], BF16, tag="cosb")
            sinb = cp.tile([128, S], BF16, tag="sinb")
            trib = cp.tile([128, 128], BF16, tag="trib")
            permb = cp.tile([128, 128], BF16, tag="permb")
            identb = cp.tile([128, 64], BF16, tag="identb")
            selAb = cp.tile([128, 512], BF16, tag="selAb")
            selBb = cp.tile([128, 512], BF16, tag="selBb")
            nc.gpsimd.memset(denomA[:], 1.0)
            nc.gpsimd.memset(denomB[:], 1.0)
            for g in range(2):
                for i in range(NST):
                    nc.vector.memset(vo[g][i][:, 64:65], 1.0)

            # ---------- Phase A: qkv projections + RoPE + v transpose ----------
            with (
                tc.tile_pool(name="wts", bufs=1) as wp,
                tc.tile_pool(name="xin", bufs=2) as xp,
                tc.tile_pool(name="aps", bufs=3, space="PSUM") as pa,
                tc.tile_pool(name="rps", bufs=2, space="PSUM") as pr,
                tc.tile_pool(name="vps", bufs=2, space="PSUM") as pv,
                tc.tile_pool(name="ropes", bufs=4) as rs,
                tc.tile_pool(name="vsb", bufs=2) as vs,
            ):
                # chunk-0 x first (split in quarters so the first matmuls can
                # start as soon as the first quarter + weights land)
                xa0 = xp.tile([128, NKT, 512], BF16, tag="xa", name="xa0")
                x0r = xT[:, 0:512].rearrange("(k p) c -> p k c", p=128)
                nc.sync.dma_start(xa0[:, 0:4, :], x0r[:, 0:4, :])
                wqa = wp.tile([128, NKT, 8 * HD], BF16, tag="wqa")
                wka = wp.tile([128, NKT, 2 * HD], BF16, tag="wka")
                wva = wp.tile([128, NKT, 2 * HD], BF16, tag="wva")
                nc.sync.dma_start(wqa[:], wq.rearrange("(k p) c -> p k c", p=128))
                nc.sync.dma_start(wka[:], wk.rearrange("(k p) c -> p k c", p=128))
                nc.sync.dma_start(wva[:], wv.rearrange("(k p) c -> p k c", p=128))
                for qtr in range(1, 4):
                    nc.sync.dma_start(
                        xa0[:, 4 * qtr:4 * qtr + 4, :], x0r[:, 4 * qtr:4 * qtr + 4, :])
                nc.sync.dma_start(cosb[:], cosT2[:])
                nc.sync.dma_start(sinb[:], sinP[:])
                nc.sync.dma_start(trib[:], tri[:])
                nc.sync.dma_start(permb[:], permm[:])
                nc.sync.dma_start(identb[:], ident[:])
                nc.sync.dma_start(selAb[:], selA[:])
                nc.sync.dma_start(selBb[:], selB[:])
                wqk = [wqa[:, k:k + 1, :] for k in range(NKT)]
                wkk = [wka[:, k:k + 1, :] for k in range(NKT)]
                wvk = [wva[:, k:k + 1, :] for k in range(NKT)]

                for n in range(NCHUNK):
                    ncol = slice(n * 512, (n + 1) * 512)
                    if n == 0:
                        xa = xa0
                    else:
                        xa = xp.tile([128, NKT, 512], BF16, tag="xa", name=f"xa{n}")
                        nc.sync.dma_start(
                            xa[:], xT[:, ncol].rearrange("(k p) c -> p k c", p=128))
                    xk = [xa[:, k:k + 1, :] for k in range(NKT)]
                    # two passes of 3 accumulators each
                    # pass A: q tiles 0,1,2 ; pass B: q tile 3, k, v
                    accsA = [pa.tile([128, 512], F32, tag="acc", name=f"accA{n}_{m}") for m in range(3)]
                    for k in range(NKT):
                        st, sp = k == 0, k == NKT - 1
                        for m in range(3):
                            nc.tensor.matmul(
                                accsA[m][:], wqk[k][:, :, m * 128:(m + 1) * 128],
                                xk[k], start=st, stop=sp)
                    accsB = [pa.tile([128, 512], F32, tag="acc", name=f"accB{n}_{m}") for m in range(3)]
                    for k in range(NKT):
                        st, sp = k == 0, k == NKT - 1
                        nc.tensor.matmul(accsB[0][:], wqk[k][:, :, 384:512], xk[k], start=st, stop=sp)
                        nc.tensor.matmul(accsB[1][:], wkk[k], xk[k], start=st, stop=sp)
                        nc.tensor.matmul(accsB[2][:], wvk[k], xk[k], start=st, stop=sp)

                    # RoPE for q0..q3 and k: dst = (acc*cos) + Pm @ (acc*sinP)
                    rope_list = [
                        (qT[0], accsA[0]), (qT[1], accsA[1]), (qT[2], accsA[2]),
                        (qT[3], accsB[0]), (kT, accsB[1]),
                    ]
                    for dst, acc in rope_list:
                        u = rs.tile([128, 512], BF16, tag="u")
                        tq = rs.tile([128, 512], BF16, tag="tq")
                        nc.vector.tensor_mul(u[:], acc[:], sinb[:, ncol])
                        nc.vector.tensor_mul(tq[:], acc[:], cosb[:, ncol])
                        rp = pr.tile([128, 512], F32, tag="rp")
                        nc.tensor.matmul(rp[:], permb[:], u[:], start=True, stop=True)
                        nc.vector.tensor_add(dst[:, ncol], tq[:], rp[:])

                    # v: evacuate + transpose into vo tiles
                    vc = vs.tile([128, 512], BF16, tag="vc")
                    nc.vector.tensor_copy(vc[:], accsB[2][:])
                    for g in range(2):
                        for blk in range(4):
                            vps = pv.tile([128, 64], BF16, tag="vps")
                            nc.tensor.transpose(
                                vps[:], vc[g * 64:(g + 1) * 64, blk * 128:(blk + 1) * 128],
                                identb[g * 64:(g + 1) * 64, :])
                            nc.vector.tensor_copy(vo[g][4 * n + blk][:, 0:64], vps[:])

            # ---------- Phase D+E: attention, normalize, o_proj per chunk ----------
            with (
                tc.tile_pool(name="wop", bufs=1) as wop,
                tc.tile_pool(name="ssp", bufs=2, space="PSUM") as sp_,
                tc.tile_pool(name="avp", bufs=2, space="PSUM") as ap_,
                tc.tile_pool(name="opp", bufs=2, space="PSUM") as op_,
                tc.tile_pool(name="esb", bufs=6) as ep,
                tc.tile_pool(name="rcb", bufs=2) as rb,
                tc.tile_pool(name="oob", bufs=3) as ob,
            ):
                woa = wop.tile([128, 4, S], BF16, tag="woa")
                nc.sync.dma_start(woa[:], wo.rearrange("(k p) c -> p k c", p=128))

                def d_group(t, j):
                    jcol = slice(j * 512, (j + 1) * 512)
                    avs = [
                        ap_.tile([65, 512], F32, tag="av", name=f"av{t}_{j}_{s_}")
                        for s_ in range(2)
                    ]
                    nblk = 4 * j + 4
                    for i in range(nblk):
                        c0 = max(0, 128 * (i - 4 * j))
                        icol = slice(i * 128, (i + 1) * 128)
                        qcol = slice(j * 512 + c0, (j + 1) * 512)
                        ss = sp_.tile([128, 2, 512], F32, tag="ss", name=f"ss{t}_{j}_{i}")
                        nc.tensor.matmul(
                            ss[:, 0:1, c0:512], kT[0:64, icol], qT[t][0:64, qcol],
                            start=True, stop=True)
                        nc.tensor.matmul(
                            ss[:, 1:2, c0:512], kT[64:128, icol], qT[t][64:128, qcol],
                            start=True, stop=True)
                        es = ep.tile([128, 2, 512], BF16, tag="es", name=f"es{t}_{j}_{i}")
                        nc.scalar.activation(
                            es[:, :, c0:512], ss[:, :, c0:512], EXP, scale=SCALE)
                        if i >= 4 * j:
                            for s_ in range(2):
                                nc.vector.tensor_mul(
                                    es[:, s_:s_ + 1, c0:c0 + 128],
                                    es[:, s_:s_ + 1, c0:c0 + 128],
                                    trib[:])
                        for s_ in range(2):
                            nc.tensor.matmul(
                                avs[s_][:, c0:512], vo[s_][i],
                                es[:, s_:s_ + 1, c0:512],
                                start=(i == 0), stop=(i == nblk - 1))
                    for s_ in range(2):
                        nc.vector.tensor_copy(
                            outT[t][64 * s_:64 * s_ + 64, jcol], avs[s_][0:64, :])
                        dn = denomA if s_ == 0 else denomB
                        nc.vector.tensor_copy(
                            dn[32 * t:32 * t + 1, jcol], avs[s_][64:65, :])

                def e_chunk(j):
                    jcol = slice(j * 512, (j + 1) * 512)
                    # normalization for chunk j (denominators final now)
                    rcpbs = []
                    for dn in (denomA, denomB):
                        rcpf = rb.tile([128, 512], F32, tag="rcpf")
                        rcpb = rb.tile([128, 512], BF16, tag="rcpb")
                        nc.vector.reciprocal_approx_fast(rcpf[:], dn[:, jcol])
                        nc.vector.tensor_copy(rcpb[:], rcpf[:])
                        rcpbs.append(rcpb)
                    for t in range(4):
                        tsl = slice(t * 128, (t + 1) * 128)
                        bcs = op_.tile([128, 512], F32, tag="op", name=f"bcs{j}_{t}")
                        nc.tensor.matmul(
                            bcs[:], selAb[:, tsl], rcpbs[0][:], start=True, stop=False)
                        nc.tensor.matmul(
                            bcs[:], selBb[:, tsl], rcpbs[1][:], start=False, stop=True)
                        for sb in range(4):
                            scol = slice(j * 512 + sb * 128, j * 512 + sb * 128 + 128)
                            nc.vector.tensor_mul(
                                outT[t][:, scol], outT[t][:, scol],
                                bcs[:, sb * 128:sb * 128 + 128])
                    # o_proj for chunk j
                    for st in range(4 * j, 4 * j + 4):
                        stcol = slice(st * 128, (st + 1) * 128)
                        oo3 = ob.tile([128, 4, 512], F32, tag="oo", name=f"oo{st}")
                        for dch in range(NCHUNK):
                            ops = op_.tile([128, 512], F32, tag="op", name=f"ops{st}_{dch}")
                            for kt in range(4):
                                nc.tensor.matmul(
                                    ops[:], outT[kt][:, stcol],
                                    woa[:, kt:kt + 1, dch * 512:(dch + 1) * 512],
                                    start=(kt == 0), stop=(kt == 3))
                            nc.vector.tensor_copy(oo3[:, dch:dch + 1, :], ops[:])
                        nc.sync.dma_start(opart[stcol, :], oo3[:])

                # emission order: stagger e(j) after the first d-group of j+1
                emission = []
                for j in range(NCHUNK):
                    emission.extend(("d", t, j) for t in range(4))
                    emission.append(("e", j))
                for idx in range(len(emission) - 1):
                    if emission[idx][0] == "e" and emission[idx + 1][0] == "d":
                        emission[idx], emission[idx + 1] = emission[idx + 1], emission[idx]
                for step in emission:
                    if step[0] == "d":
                        d_group(step[1], step[2])
                    else:
                        e_chunk(step[1])

    nc.compile()
    return nc


_PROGRAM = None


def _get_program():
    global _PROGRAM
    if _PROGRAM is None:
        _PROGRAM = _build_program()
    return _PROGRAM


def _make_in_maps(x, cos, sin, Wq, Wk, Wv, Wo):
    x = np.asarray(x, np.float32)
    cos = np.asarray(cos, np.float32)
    sin = np.asarray(sin, np.float32)
    Wq, Wk, Wv, Wo = (np.asarray(w, np.float32) for w in (Wq, Wk, Wv, Wo))

    cosT = cos.T  # [64, S]
    sinT = sin.T
    cosT2 = np.tile(cosT, (2, 1)).astype(BF)
    # s''[i] = sign(sigma(i)) * sin[sigma(i)], sigma = xor-32 within each 64
    stld = np.concatenate([-sinT[:32], sinT[32:]], 0)          # s~ (signed sin)
    sPrm = np.concatenate([stld[32:64], stld[0:32]], 0)        # s'' = s~ . sigma
    sinP = np.tile(sPrm, (2, 1)).astype(BF)
    tri = (np.arange(128)[None, :] >= np.arange(128)[:, None]).astype(BF)
    # permutation matmul matrix: out[p] = u[sigma(p)];  Pm[r, p] = 1 iff r = sigma(p)
    sig = np.arange(128)
    sig = (sig // 64) * 64 + ((sig % 64) + 32) % 64
    permm = np.zeros((128, 128), np.float32)
    permm[sig, np.arange(128)] = 1.0
    permm = permm.astype(BF)
    ident = np.tile(np.eye(64, dtype=np.float32), (2, 1)).astype(BF)
    # selectors: bcs[p<64, s] = rcpA[32t, s]; bcs[p>=64, s] = rcpB[32t, s]
    selA = np.zeros((128, 512), np.float32)
    selB = np.zeros((128, 512), np.float32)
    for t in range(4):
        selA[32 * t, 128 * t:128 * t + 64] = 1.0
        selB[32 * t, 128 * t + 64:128 * t + 128] = 1.0
    selA = selA.astype(BF)
    selB = selB.astype(BF)

    perm = [0, 4, 1, 5, 2, 6, 3, 7]
    in_maps = []
    for c in range(8):
        b, q = c // 4, c % 4
        idx = np.concatenate([np.arange(HD) + (8 * q + j) * HD for j in perm])
        in_maps.append({
            "xT": np.ascontiguousarray(x[b].T).astype(BF),
            "wq": np.ascontiguousarray(Wq[:, idx]).astype(BF),
            "wk": np.ascontiguousarray(Wk[:, 2 * q * HD:(2 * q + 2) * HD]).astype(BF),
            "wv": np.ascontiguousarray(Wv[:, 2 * q * HD:(2 * q + 2) * HD]).astype(BF),
            "wo": np.ascontiguousarray(Wo[idx, :]).astype(BF),
            "cosT2": cosT2,
            "sinP": sinP,
            "tri": tri,
            "permm": permm,
            "ident": ident,
            "selA": selA,
            "selB": selB,
        })
    return in_maps


def _execute(in_maps, trace=False):
    nc = _get_program()
    return bass_utils.run_bass_kernel_spmd(
        nc, in_maps, core_ids=list(range(8)), trace=trace)


def kernel(x, cos, sin, Wq, Wk, Wv, Wo):
    in_maps = _make_in_maps(x, cos, sin, Wq, Wk, Wv, Wo)
    res = _execute(in_maps, trace=False)
    parts = [r["opart"] for r in res.results]
    out = np.empty((B, S, D), dtype=np.float32)
    for b in range(B):
        p = parts[4 * b:4 * b + 4]
        out[b] = (p[0] + p[1]) + (p[2] + p[3])
    return out
